# revision 10
# baseline (speedup 1.0000x reference)
"""MoE feed-forward (top-2 routing, E=8 experts) on 8 TRN2 NeuronCores.

Strategy: expert parallelism with host-side routing/dispatch.
  - Host computes the router (softmax + top-2 + renormalize) in float64,
    gathers each expert's tokens (padded to a common capacity Cx), and
    pre-tiles/pre-transposes all operands into DMA-friendly layouts.
  - Core e runs the GLU MLP for expert e over its Cx gathered tokens,
    everything in transposed [feature, token] orientation:
      phase 1: hT[H, Cx] = silu(w1[e] @ xT) * (w3[e] @ xT)   (x resident,
               w1/w3 streamed once, hT kept resident in SBUF as bf16)
      phase 2: yT[D, Cx] = w2[e] @ hT, scaled per-token (along the free
               axis, via a partition-broadcast scale row) by the
               renormalized routing weight (w2 streamed once)
  - Host scatter-adds the 8 per-expert outputs into the final [T, D].

Matmuls run in bf16 (1 cyc/row on the PE vs 4 for fp32) with fp32 PSUM
accumulation. The token axis always rides the matmul free dimension, so
compute scales with the exact token count Cx, not a 128-padded capacity.
"""
import sys

if "/opt/trn_rl_repo" not in sys.path:
    sys.path.insert(0, "/opt/trn_rl_repo")

import numpy as np
import ml_dtypes

import concourse.bass as bass
import concourse.mybir as mybir
from concourse import bacc
from concourse.tile import TileContext
from concourse.bass_utils import run_bass_kernel_spmd

BF16 = ml_dtypes.bfloat16
P = 128
D = 2048   # model dim
H = 4096   # hidden dim
E = 8      # experts == cores
TOP_K = 2
DO = D // P   # 16 contraction chunks for layer 1
HO = H // P   # 32 contraction chunks for layer 2
DT = D // P   # 16 output-row tiles for phase 2


def _route(x, router_w):
    """Top-2 expert selection + renormalized weights (float64 host math)."""
    logits = x.astype(np.float64) @ router_w.astype(np.float64).T
    m = logits.max(axis=1, keepdims=True)
    p = np.exp(logits - m)
    p /= p.sum(axis=1, keepdims=True)
    sel = np.argsort(-p, axis=1, kind="stable")[:, :TOP_K]
    rw = np.take_along_axis(p, sel, axis=1)
    rw /= rw.sum(axis=1, keepdims=True)
    return sel, rw.astype(np.float32)


def _tblocks(Cx):
    """Token blocks: a small first block (shrinks the startup DMA wall),
    then roughly equal blocks of <=512; all blocks >=256 when Cx >= 512."""
    if Cx <= 512:
        return [(0, Cx)]
    rem = Cx - 256
    n = -(-rem // 512)
    base = rem // n
    sizes = [256] + [base + (1 if i < rem - base * n else 0) for i in range(n)]
    out, t0 = [], 0
    for s in sizes:
        out.append((t0, s))
        t0 += s
    return out


def _build(Cx):
    """Bass program: one expert's GLU MLP over Cx gathered tokens."""
    f32 = mybir.dt.float32
    bf16 = mybir.dt.bfloat16

    nc = bacc.Bacc(None, target_bir_lowering=False)
    xthd = nc.dram_tensor("xthd", [P, DO, Cx], bf16, kind="ExternalInput")
    w1thd = nc.dram_tensor("w1thd", [P, HO, DO, P], bf16, kind="ExternalInput")
    w3thd = nc.dram_tensor("w3thd", [P, HO, DO, P], bf16, kind="ExternalInput")
    w2thd = nc.dram_tensor("w2thd", [P, DT, HO, P], bf16, kind="ExternalInput")
    csx = nc.dram_tensor("csx", [Cx], f32, kind="ExternalInput")
    outT = nc.dram_tensor("outT", [D, Cx], f32, kind="ExternalOutput")
    out_v = outT.rearrange("(dt p) c -> p dt c", p=P)

    tblocks = _tblocks(Cx)
    Silu = mybir.ActivationFunctionType.Silu

    with TileContext(nc) as tc:
        with (
            tc.tile_pool(name="resident", bufs=1) as resident,
            tc.tile_pool(name="w13", bufs=3) as w13pool,
            tc.tile_pool(name="silu", bufs=4) as silupool,
            tc.tile_pool(name="w2", bufs=6) as w2pool,
            tc.tile_pool(name="y", bufs=4) as ypool,
            tc.tile_pool(name="ps13", bufs=2, space="PSUM") as ps13,
            tc.tile_pool(name="ps2", bufs=4, space="PSUM") as ps2,
        ):
            xsb = resident.tile([P, DO, Cx], bf16, name="xsb")
            hsb = resident.tile([P, HO, Cx], bf16, name="hsb")
            cbb = resident.tile([P, Cx], f32, name="cbb")

            # Startup-critical DMAs: the dk-th matmul of the first psum
            # group needs only w1[ht=0, dk] + x[dk, block0]. Emit those as
            # per-dk chunk pairs round-robined over four engine queues so
            # the first matmul fires after ~96 KiB instead of ~2 MiB and
            # the rest stream in parallel with compute.
            t00, tn0 = tblocks[0]
            qs = [nc.sync, nc.scalar, nc.gpsimd]
            w1t0 = w13pool.tile([P, DO, P], bf16, name="w1t")
            w3t0 = w13pool.tile([P, DO, P], bf16, name="w3t")
            for dk in range(DO):
                eng = qs[dk % len(qs)]
                eng.dma_start(out=w1t0[:, dk, :], in_=w1thd[:, 0, dk, :])
                eng.dma_start(
                    out=xsb[:, dk, t00 : t00 + tn0], in_=xthd[:, dk, t00 : t00 + tn0]
                )
            for dk in range(DO):
                qs[dk % len(qs)].dma_start(
                    out=w3t0[:, dk, :], in_=w3thd[:, 0, dk, :]
                )
            w13_0 = [w1t0, w3t0]
            # Bulk of x on the gpsimd queue so it doesn't block the
            # sync-queue weight stream; ditto the broadcast scale row.
            for (t0, tn) in tblocks[1:]:
                for do in range(DO):
                    nc.gpsimd.dma_start(
                        out=xsb[:, do, t0 : t0 + tn], in_=xthd[:, do, t0 : t0 + tn]
                    )
            csx_ap = csx[:]
            csx_bcast = bass.AP(
                tensor=csx_ap.tensor, offset=csx_ap.offset, ap=[[0, P], *csx_ap.ap]
            )
            nc.gpsimd.dma_start(out=cbb[:], in_=csx_bcast)

            # ---- phase 1: hT = silu(w1 xT) * (w3 xT), laid out [h, t] ----
            for ht in range(HO):
                if ht == 0:
                    w1t, w3t = w13_0
                else:
                    w1t = w13pool.tile([P, DO, P], bf16, name="w1t")
                    nc.sync.dma_start(out=w1t[:], in_=w1thd[:, ht, :, :])
                    w3t = w13pool.tile([P, DO, P], bf16, name="w3t")
                    nc.sync.dma_start(out=w3t[:], in_=w3thd[:, ht, :, :])
                for (t0, tn) in tblocks:
                    pg = ps13.tile([P, 512], f32, name="pg")[:, :tn]
                    pu = ps13.tile([P, 512], f32, name="pu")[:, :tn]
                    for dk in range(DO):
                        nc.tensor.matmul(
                            pg, w1t[:, dk, :], xsb[:, dk, t0 : t0 + tn],
                            start=(dk == 0), stop=(dk == DO - 1),
                        )
                    for dk in range(DO):
                        nc.tensor.matmul(
                            pu, w3t[:, dk, :], xsb[:, dk, t0 : t0 + tn],
                            start=(dk == 0), stop=(dk == DO - 1),
                        )
                    st = silupool.tile([P, 512], f32, name="st")[:, :tn]
                    nc.scalar.activation(st, pg, Silu)
                    nc.vector.tensor_mul(hsb[:, ht, t0 : t0 + tn], st, pu)

            # ---- phase 2: yT[d, t] = sum_h w2t[h, d] * hT[h, t], scaled ----
            G = 8  # ho-tiles per w2 DMA (2 KiB contiguous per partition)
            for dt in range(DT):
                w2gs = []
                for g in range(HO // G):
                    w2g = w2pool.tile([P, G, P], bf16, name="w2g")
                    nc.sync.dma_start(
                        out=w2g[:], in_=w2thd[:, dt, g * G : (g + 1) * G, :]
                    )
                    w2gs.append(w2g)
                for (t0, tn) in tblocks:
                    py = ps2.tile([P, 512], f32, name="py")[:, :tn]
                    for ho in range(HO):
                        nc.tensor.matmul(
                            py, w2gs[ho // G][:, ho % G, :],
                            hsb[:, ho, t0 : t0 + tn],
                            start=(ho == 0), stop=(ho == HO - 1),
                        )
                    ysb = ypool.tile([P, 512], f32, name="ysb")[:, :tn]
                    nc.vector.tensor_mul(ysb, py, cbb[:, t0 : t0 + tn])
                    nc.sync.dma_start(out=out_v[:, dt, t0 : t0 + tn], in_=ysb)

    nc.compile()
    return nc


def _prep_core(x, w1_e, w3_e, w2_e, idx, cw, Cx):
    """Per-core input arrays in device layouts (see _build docstring)."""
    cnt = len(idx)
    xg = np.zeros((Cx, D), np.float32)
    xg[:cnt] = x[idx]
    # [p, do, c] with d = do*P + p
    xthd = np.ascontiguousarray(
        xg.T.reshape(DO, P, Cx).transpose(1, 0, 2).astype(BF16)
    )
    # [p, ht, do, hi] with d = do*P + p, h = ht*P + hi  (from w1 [H, D])
    w1thd = np.ascontiguousarray(
        w1_e.reshape(HO, P, DO, P).transpose(3, 0, 2, 1).astype(BF16)
    )
    w3thd = np.ascontiguousarray(
        w3_e.reshape(HO, P, DO, P).transpose(3, 0, 2, 1).astype(BF16)
    )
    # [p, dt, ho, di] with h = ho*P + p, d = dt*P + di  (from w2 [D, H])
    w2thd = np.ascontiguousarray(
        w2_e.reshape(DT, P, HO, P).transpose(3, 0, 2, 1).astype(BF16)
    )
    csf = np.zeros(Cx, np.float32)
    csf[:cnt] = cw
    return {
        "xthd": xthd, "w1thd": w1thd, "w3thd": w3thd,
        "w2thd": w2thd, "csx": csf,
    }


def kernel(x, router_w, w1, w3, w2, _trace=False):
    T = x.shape[0]
    x = np.asarray(x, np.float32)
    router_w = np.asarray(router_w, np.float32)
    w1 = np.asarray(w1, np.float32)
    w3 = np.asarray(w3, np.float32)
    w2 = np.asarray(w2, np.float32)
    assert x.shape[1] == D and router_w.shape == (E, D), (x.shape, router_w.shape)
    assert w1.shape == w3.shape == (E, H, D) and w2.shape == (E, D, H)

    sel, rw = _route(x, router_w)
    idxs, cws = [], []
    for e in range(E):
        mask = sel == e  # [T, 2]; a token never selects the same expert twice
        tok = np.nonzero(mask.any(axis=1))[0]
        cw = np.where(mask[tok, 0], rw[tok, 0], rw[tok, 1])
        idxs.append(tok)
        cws.append(cw)

    # One work chunk per (expert, token-slice). For the expected token
    # distribution (~T*K/E per expert) this is exactly one chunk per core;
    # the chunking is a safety net for pathological routing skew, since
    # SBUF residency caps the per-core token span at CMAX.
    CMAX = 1280
    chunks = []  # (expert, token-index array, weight array)
    for e in range(E):
        tok, cw = idxs[e], cws[e]
        for s in range(0, max(len(tok), 1), CMAX):
            chunks.append((e, tok[s : s + CMAX], cw[s : s + CMAX]))

    mx = max(len(t) for _, t, _ in chunks)
    Cx = max(16, -(-mx // 8) * 8)  # exact span, multiple of 8

    nc = _build(Cx)
    out = np.zeros((T, D), np.float32)
    exec_ns = []
    for b in range(0, len(chunks), E):
        batch = chunks[b : b + E]
        while len(batch) < E:  # pad the SPMD batch with zero-weight work
            batch.append((0, idxs[0][:0], cws[0][:0]))
        in_maps = [
            _prep_core(x, w1[e], w3[e], w2[e], tok, cw, Cx)
            for (e, tok, cw) in batch
        ]
        res = run_bass_kernel_spmd(
            nc, in_maps, core_ids=list(range(E)), trace=_trace
        )
        for i, (e, tok, cw) in enumerate(batch):
            if len(tok):
                out[tok] += res.results[i]["outT"][:, : len(tok)].T
        if _trace:
            exec_ns.append(res.exec_time_ns)
            kernel.last_results = res
    if _trace:
        kernel.last_exec_time_ns = sum(filter(None, exec_ns)) or None
    return out


# revision 11
# speedup vs baseline: 1.0047x; 1.0047x over previous
"""MoE feed-forward (top-2 routing, E=8 experts) on 8 TRN2 NeuronCores.

Strategy: expert parallelism with host-side routing/dispatch.
  - Host computes the router (softmax + top-2 + renormalize) in float64,
    gathers each expert's tokens (padded to a common capacity Cx), and
    pre-tiles/pre-transposes all operands into DMA-friendly layouts.
  - Core e runs the GLU MLP for expert e over its Cx gathered tokens,
    everything in transposed [feature, token] orientation:
      phase 1: hT[H, Cx] = silu(w1[e] @ xT) * (w3[e] @ xT)   (x resident,
               w1/w3 streamed once, hT kept resident in SBUF as bf16)
      phase 2: yT[D, Cx] = w2[e] @ hT, scaled per-token (along the free
               axis, via a partition-broadcast scale row) by the
               renormalized routing weight (w2 streamed once)
  - Host scatter-adds the 8 per-expert outputs into the final [T, D].

Matmuls run in bf16 (1 cyc/row on the PE vs 4 for fp32) with fp32 PSUM
accumulation. The token axis always rides the matmul free dimension, so
compute scales with the exact token count Cx, not a 128-padded capacity.
"""
import sys

if "/opt/trn_rl_repo" not in sys.path:
    sys.path.insert(0, "/opt/trn_rl_repo")

import numpy as np
import ml_dtypes

import concourse.bass as bass
import concourse.mybir as mybir
from concourse import bacc
from concourse.tile import TileContext
from concourse.bass_utils import run_bass_kernel_spmd

BF16 = ml_dtypes.bfloat16
P = 128
D = 2048   # model dim
H = 4096   # hidden dim
E = 8      # experts == cores
TOP_K = 2
DO = D // P   # 16 contraction chunks for layer 1
HO = H // P   # 32 contraction chunks for layer 2
DT = D // P   # 16 output-row tiles for phase 2


def _route(x, router_w):
    """Top-2 expert selection + renormalized weights (float64 host math)."""
    logits = x.astype(np.float64) @ router_w.astype(np.float64).T
    m = logits.max(axis=1, keepdims=True)
    p = np.exp(logits - m)
    p /= p.sum(axis=1, keepdims=True)
    sel = np.argsort(-p, axis=1, kind="stable")[:, :TOP_K]
    rw = np.take_along_axis(p, sel, axis=1)
    rw /= rw.sum(axis=1, keepdims=True)
    return sel, rw.astype(np.float32)


def _tblocks(Cx):
    """Token blocks: a small first block (shrinks the startup DMA wall),
    then roughly equal blocks of <=512; all blocks >=256 when Cx >= 512."""
    if Cx <= 512:
        return [(0, Cx)]
    rem = Cx - 256
    n = -(-rem // 512)
    base = rem // n
    sizes = [256] + [base + (1 if i < rem - base * n else 0) for i in range(n)]
    out, t0 = [], 0
    for s in sizes:
        out.append((t0, s))
        t0 += s
    return out


def _build(Cx):
    """Bass program: one expert's GLU MLP over Cx gathered tokens."""
    f32 = mybir.dt.float32
    bf16 = mybir.dt.bfloat16

    nc = bacc.Bacc(None, target_bir_lowering=False)
    xthd = nc.dram_tensor("xthd", [P, DO, Cx], bf16, kind="ExternalInput")
    w1thd = nc.dram_tensor("w1thd", [P, HO, DO, P], bf16, kind="ExternalInput")
    w3thd = nc.dram_tensor("w3thd", [P, HO, DO, P], bf16, kind="ExternalInput")
    w2thd = nc.dram_tensor("w2thd", [P, DT, HO, P], bf16, kind="ExternalInput")
    csx = nc.dram_tensor("csx", [Cx], f32, kind="ExternalInput")
    outT = nc.dram_tensor("outT", [D, Cx], f32, kind="ExternalOutput")
    out_v = outT.rearrange("(dt p) c -> p dt c", p=P)

    tblocks = _tblocks(Cx)
    Silu = mybir.ActivationFunctionType.Silu

    with TileContext(nc) as tc:
        with (
            tc.tile_pool(name="resident", bufs=1) as resident,
            tc.tile_pool(name="w13", bufs=3) as w13pool,
            tc.tile_pool(name="silu", bufs=4) as silupool,
            tc.tile_pool(name="w2", bufs=6) as w2pool,
            tc.tile_pool(name="y", bufs=4) as ypool,
            tc.tile_pool(name="ps13", bufs=2, space="PSUM") as ps13,
            tc.tile_pool(name="ps2", bufs=4, space="PSUM") as ps2,
        ):
            xsb = resident.tile([P, DO, Cx], bf16, name="xsb")
            hsb = resident.tile([P, HO, Cx], bf16, name="hsb")
            cbb = resident.tile([P, Cx], f32, name="cbb")

            # Startup-critical DMAs: the dk-th matmul of the first psum
            # group needs only w1[ht=0, dk] + x[dk, block0]. Emit those as
            # per-dk chunk pairs round-robined over four engine queues so
            # the first matmul fires after ~96 KiB instead of ~2 MiB and
            # the rest stream in parallel with compute.
            t00, tn0 = tblocks[0]
            w1t0 = w13pool.tile([P, DO, P], bf16, name="w1t")
            w3t0 = w13pool.tile([P, DO, P], bf16, name="w3t")
            nc.sync.dma_start(out=w1t0[:], in_=w1thd[:, 0, :, :])
            nc.scalar.dma_start(
                out=xsb[:, : DO // 2, t00 : t00 + tn0],
                in_=xthd[:, : DO // 2, t00 : t00 + tn0],
            )
            nc.gpsimd.dma_start(
                out=xsb[:, DO // 2 :, t00 : t00 + tn0],
                in_=xthd[:, DO // 2 :, t00 : t00 + tn0],
            )
            nc.sync.dma_start(out=w3t0[:], in_=w3thd[:, 0, :, :])
            w13_0 = [w1t0, w3t0]
            # Bulk of x on the gpsimd queue so it doesn't block the
            # sync-queue weight stream; ditto the broadcast scale row.
            for (t0, tn) in tblocks[1:]:
                for do in range(DO):
                    nc.gpsimd.dma_start(
                        out=xsb[:, do, t0 : t0 + tn], in_=xthd[:, do, t0 : t0 + tn]
                    )
            csx_ap = csx[:]
            csx_bcast = bass.AP(
                tensor=csx_ap.tensor, offset=csx_ap.offset, ap=[[0, P], *csx_ap.ap]
            )
            nc.gpsimd.dma_start(out=cbb[:], in_=csx_bcast)

            # ---- phase 1: hT = silu(w1 xT) * (w3 xT), laid out [h, t] ----
            for ht in range(HO):
                if ht == 0:
                    w1t, w3t = w13_0
                else:
                    w1t = w13pool.tile([P, DO, P], bf16, name="w1t")
                    nc.sync.dma_start(out=w1t[:], in_=w1thd[:, ht, :, :])
                    w3t = w13pool.tile([P, DO, P], bf16, name="w3t")
                    nc.sync.dma_start(out=w3t[:], in_=w3thd[:, ht, :, :])
                for (t0, tn) in tblocks:
                    pg = ps13.tile([P, 512], f32, name="pg")[:, :tn]
                    pu = ps13.tile([P, 512], f32, name="pu")[:, :tn]
                    for dk in range(DO):
                        nc.tensor.matmul(
                            pg, w1t[:, dk, :], xsb[:, dk, t0 : t0 + tn],
                            start=(dk == 0), stop=(dk == DO - 1),
                        )
                    for dk in range(DO):
                        nc.tensor.matmul(
                            pu, w3t[:, dk, :], xsb[:, dk, t0 : t0 + tn],
                            start=(dk == 0), stop=(dk == DO - 1),
                        )
                    st = silupool.tile([P, 512], f32, name="st")[:, :tn]
                    nc.scalar.activation(st, pg, Silu)
                    nc.vector.tensor_mul(hsb[:, ht, t0 : t0 + tn], st, pu)

            # ---- phase 2: yT[d, t] = sum_h w2t[h, d] * hT[h, t], scaled ----
            G = 8  # ho-tiles per w2 DMA (2 KiB contiguous per partition)
            for dt in range(DT):
                w2gs = []
                for g in range(HO // G):
                    w2g = w2pool.tile([P, G, P], bf16, name="w2g")
                    nc.sync.dma_start(
                        out=w2g[:], in_=w2thd[:, dt, g * G : (g + 1) * G, :]
                    )
                    w2gs.append(w2g)
                for (t0, tn) in tblocks:
                    py = ps2.tile([P, 512], f32, name="py")[:, :tn]
                    for ho in range(HO):
                        nc.tensor.matmul(
                            py, w2gs[ho // G][:, ho % G, :],
                            hsb[:, ho, t0 : t0 + tn],
                            start=(ho == 0), stop=(ho == HO - 1),
                        )
                    ysb = ypool.tile([P, 512], f32, name="ysb")[:, :tn]
                    nc.vector.tensor_mul(ysb, py, cbb[:, t0 : t0 + tn])
                    nc.sync.dma_start(out=out_v[:, dt, t0 : t0 + tn], in_=ysb)

    nc.compile()
    return nc


def _prep_core(x, w1_e, w3_e, w2_e, idx, cw, Cx):
    """Per-core input arrays in device layouts (see _build docstring)."""
    cnt = len(idx)
    xg = np.zeros((Cx, D), np.float32)
    xg[:cnt] = x[idx]
    # [p, do, c] with d = do*P + p
    xthd = np.ascontiguousarray(
        xg.T.reshape(DO, P, Cx).transpose(1, 0, 2).astype(BF16)
    )
    # [p, ht, do, hi] with d = do*P + p, h = ht*P + hi  (from w1 [H, D])
    w1thd = np.ascontiguousarray(
        w1_e.reshape(HO, P, DO, P).transpose(3, 0, 2, 1).astype(BF16)
    )
    w3thd = np.ascontiguousarray(
        w3_e.reshape(HO, P, DO, P).transpose(3, 0, 2, 1).astype(BF16)
    )
    # [p, dt, ho, di] with h = ho*P + p, d = dt*P + di  (from w2 [D, H])
    w2thd = np.ascontiguousarray(
        w2_e.reshape(DT, P, HO, P).transpose(3, 0, 2, 1).astype(BF16)
    )
    csf = np.zeros(Cx, np.float32)
    csf[:cnt] = cw
    return {
        "xthd": xthd, "w1thd": w1thd, "w3thd": w3thd,
        "w2thd": w2thd, "csx": csf,
    }


def kernel(x, router_w, w1, w3, w2, _trace=False):
    T = x.shape[0]
    x = np.asarray(x, np.float32)
    router_w = np.asarray(router_w, np.float32)
    w1 = np.asarray(w1, np.float32)
    w3 = np.asarray(w3, np.float32)
    w2 = np.asarray(w2, np.float32)
    assert x.shape[1] == D and router_w.shape == (E, D), (x.shape, router_w.shape)
    assert w1.shape == w3.shape == (E, H, D) and w2.shape == (E, D, H)

    sel, rw = _route(x, router_w)
    idxs, cws = [], []
    for e in range(E):
        mask = sel == e  # [T, 2]; a token never selects the same expert twice
        tok = np.nonzero(mask.any(axis=1))[0]
        cw = np.where(mask[tok, 0], rw[tok, 0], rw[tok, 1])
        idxs.append(tok)
        cws.append(cw)

    # One work chunk per (expert, token-slice). For the expected token
    # distribution (~T*K/E per expert) this is exactly one chunk per core;
    # the chunking is a safety net for pathological routing skew, since
    # SBUF residency caps the per-core token span at CMAX.
    CMAX = 1280
    chunks = []  # (expert, token-index array, weight array)
    for e in range(E):
        tok, cw = idxs[e], cws[e]
        for s in range(0, max(len(tok), 1), CMAX):
            chunks.append((e, tok[s : s + CMAX], cw[s : s + CMAX]))

    mx = max(len(t) for _, t, _ in chunks)
    Cx = max(16, -(-mx // 8) * 8)  # exact span, multiple of 8

    nc = _build(Cx)
    out = np.zeros((T, D), np.float32)
    exec_ns = []
    for b in range(0, len(chunks), E):
        batch = chunks[b : b + E]
        while len(batch) < E:  # pad the SPMD batch with zero-weight work
            batch.append((0, idxs[0][:0], cws[0][:0]))
        in_maps = [
            _prep_core(x, w1[e], w3[e], w2[e], tok, cw, Cx)
            for (e, tok, cw) in batch
        ]
        res = run_bass_kernel_spmd(
            nc, in_maps, core_ids=list(range(E)), trace=_trace
        )
        for i, (e, tok, cw) in enumerate(batch):
            if len(tok):
                out[tok] += res.results[i]["outT"][:, : len(tok)].T
        if _trace:
            exec_ns.append(res.exec_time_ns)
            kernel.last_results = res
    if _trace:
        kernel.last_exec_time_ns = sum(filter(None, exec_ns)) or None
    return out
